# revision 1
# baseline (speedup 1.0000x reference)
"""v4: window-batched DMAs + fully-grouped K path (lm) + rebalanced engines.

Per window w (512 rows = 4 chunks) of sequence n:
  - ONE DMA per tensor loads [128, 4*D] (partition = row-within-chunk).
  - K path entirely grouped in LM: min/max/Exp/recombine/branch-muls at [C, 4D].
  - Q path: per-chunk PE transpose (fp32) -> PSUM-sourced min/max staged into
    [D, W] windows -> grouped Exp/recombine; qt/qtc/qts grouped in FM.
  - kf (FM) via per-chunk PE transpose of klm + copy; kcf/ksf grouped FM muls.
  - q2: per-chunk transpose + copy -> window matmul -> grouped sin pipeline.
SCAN per chunk: P (3 mm), mask, intra+inter (4 mm), states (3 mm, PSUM), epi.
Output stores batched per window.
"""

import math

import numpy as np

import concourse.bass as bass
import concourse.tile as tile
from concourse import bacc, mybir
from concourse.bass_utils import run_bass_kernel_spmd
from concourse.masks import make_identity

F32 = mybir.dt.float32
F16 = mybir.dt.float16
AF = mybir.ActivationFunctionType
OP = mybir.AluOpType

N, L, H, D = 4, 2048, 8, 128
C = 128
NCH = L // C
DV1 = D + 1
TWO_PI = 2.0 * math.pi
MAGIC = float(np.float32(1.5 * 2**23))
EPS = 1e-6

_CACHE = {}


def build_nc(n_seq=N, nch=NCH):
    l_eff = nch * C
    W = min(512, l_eff)
    nwin = l_eff // W
    cpw = W // C
    nc = bacc.Bacc(None, target_bir_lowering=False, debug=False)

    # 4D views so one DMA grabs [p, cc, d] = [128, cpw, D]
    q_ext = nc.declare_dram_parameter("queries", [n_seq, nch, C, D], F32, isOutput=False)
    q2_ext = nc.declare_dram_parameter("q2", [n_seq, nch, C, D], F32, isOutput=False)
    k_ext = nc.declare_dram_parameter("keys", [n_seq, nch, C, D], F32, isOutput=False)
    v_ext = nc.declare_dram_parameter("values", [n_seq, nch, C, D], F32, isOutput=False)
    om_ext = nc.declare_dram_parameter("omega", [D, D], F32, isOutput=False)
    mask_ext = nc.declare_dram_parameter("mask", [C, C], F16, isOutput=False)
    pos_ext = nc.declare_dram_parameter("pos2pi", [D, l_eff], F32, isOutput=False)
    out_ext = nc.declare_dram_parameter("out", [n_seq, nch, C, D], F32, isOutput=True)

    with tile.TileContext(nc) as tc:
        with (
            tc.tile_pool(name="persist", bufs=1) as pp,
            tc.tile_pool(name="seqst", bufs=2) as sq_,
            tc.tile_pool(name="win", bufs=2) as win,
            tc.tile_pool(name="nfp", bufs=4) as nfp,
            tc.tile_pool(name="vwp", bufs=4) as vwp,
            tc.tile_pool(name="drs", bufs=2, space="DRAM") as drs,
            tc.tile_pool(name="io", bufs=3) as io,
            tc.tile_pool(name="work", bufs=3) as wk,
            tc.tile_pool(name="outp", bufs=3) as op_,
            tc.tile_pool(name="ptr", bufs=2, space="PSUM") as ptr,
            tc.tile_pool(name="pq2", bufs=1, space="PSUM") as pq2,
            tc.tile_pool(name="pP", bufs=1, space="PSUM") as pP,
            tc.tile_pool(name="pO", bufs=1, space="PSUM") as pO,
            tc.tile_pool(name="pS", bufs=1, space="PSUM") as pS,
        ):
            # ---------------- one-time setup ----------------
            id16 = pp.tile([D, D], F16, tag="id16")
            make_identity(nc, id16[:])
            id32 = pp.tile([D, D], F32, tag="id32")
            make_identity(nc, id32[:])
            magic_col = pp.tile([D, 1], F32, tag="magic")
            nc.gpsimd.memset(magic_col[:], MAGIC)

            omega_sb = pp.tile([D, D], F32, tag="omega")
            nc.sync.dma_start(out=omega_sb[:], in_=om_ext[:, :])
            omega_s = pp.tile([D, D], F32, tag="omega_s")
            nc.scalar.activation(omega_s[:], omega_sb[:], AF.Copy, scale=1.0 / TWO_PI)
            mask_sb = pp.tile([C, C], F16, tag="mask")
            nc.sync.dma_start(out=mask_sb[:], in_=mask_ext[:, :])
            pos_sb = pp.tile([D, l_eff], F32, tag="pos")
            nc.sync.dma_start(out=pos_sb[:], in_=pos_ext[:, :])

            ones_col = pp.tile([D, 1], F32, tag="ones")
            nc.gpsimd.memset(ones_col[:], 1.0)
            wcol_ps = ptr.tile([D, 1], F32, tag="tr")
            nc.tensor.matmul(wcol_ps[:], omega_sb[:], ones_col[:], start=True, stop=True)
            wcol = pp.tile([D, 1], F32, tag="wcol")
            nc.vector.tensor_copy(wcol[:], wcol_ps[:])
            wcol2 = pp.tile([D, 1], F32, tag="wcol2")
            nc.scalar.activation(wcol2[:], wcol[:], AF.Copy, scale=2.0)

            def sin_pipe(dst, ysrc, pool, fd, shift=None, tagp="tp"):
                if shift is not None:
                    ys = pool.tile([D, fd], F32, tag=f"{tagp}_ys")
                    nc.vector.tensor_scalar(ys[:], ysrc[:], shift, None, OP.add)
                else:
                    ys = ysrc
                k1 = pool.tile([D, fd], F32, tag=f"{tagp}_k1")
                nc.vector.tensor_scalar(k1[:], ys[:], MAGIC, None, OP.add)
                nf = pool.tile([D, fd], F32, tag=f"{tagp}_nf")
                nc.vector.scalar_tensor_tensor(nf[:], k1[:], MAGIC, ys[:], OP.subtract, OP.subtract)
                nc.scalar.activation(dst, nf[:], AF.Sin, scale=-TWO_PI)

            # chunk-0 exact tables (fp32, [D, C]; pre-doubled)
            s2_0 = pp.tile([D, C], F32, tag="s2_0")
            c2_0 = pp.tile([D, C], F32, tag="c2_0")
            sc_0 = pp.tile([D, C], F32, tag="sc_0")
            # double-angle fm tables (fp16, [D, L]) + lm tables ([C, nch*D])
            c2t_fm = pp.tile([D, l_eff], F16, tag="c2t_fm")
            s2t_fm = pp.tile([D, l_eff], F16, tag="s2t_fm")
            c2t_lm = pp.tile([C, nch * D], F16, tag="c2t_lm")
            s2t_lm = pp.tile([C, nch * D], F16, tag="s2t_lm")
            with tc.tile_pool(name="trig", bufs=1) as tg:
                y0 = tg.tile([D, C], F32, tag="y0")
                nc.vector.tensor_scalar(y0[:], pos_sb[:, 0:C], wcol[:, 0:1], None, OP.mult)
                s_0 = tg.tile([D, C], F32, tag="s_0")
                c_0 = tg.tile([D, C], F32, tag="c_0")
                sin_pipe(s_0[:], y0, tg, C, tagp="t0a")
                sin_pipe(c_0[:], y0, tg, C, shift=0.25, tagp="t0b")
                nc.vector.scalar_tensor_tensor(s2_0[:], s_0[:], 2.0, s_0[:], OP.mult, OP.mult)
                nc.vector.scalar_tensor_tensor(c2_0[:], c_0[:], 2.0, c_0[:], OP.mult, OP.mult)
                nc.vector.scalar_tensor_tensor(sc_0[:], s_0[:], 2.0, c_0[:], OP.mult, OP.mult)
                for st in range(nwin):
                    ssl = bass.ds(st * W, W)
                    y = tg.tile([D, W], F32, tag="trig_y")
                    nc.vector.tensor_scalar(y[:], pos_sb[:, ssl], wcol2[:, 0:1], None, OP.mult)
                    sin_pipe(s2t_fm[:, ssl], y, tg, W, tagp="tda")
                    sin_pipe(c2t_fm[:, ssl], y, tg, W, shift=0.25, tagp="tdb")
            for c in range(nch):
                sl = bass.ts(c, C)
                dsl = bass.ts(c, D)
                tpc = ptr.tile([C, C], F16, tag="tr")
                nc.tensor.transpose(tpc[:], c2t_fm[:, sl], id16[:])
                nc.vector.tensor_copy(c2t_lm[:, dsl], tpc[:])
                tps = ptr.tile([C, C], F16, tag="tr")
                nc.tensor.transpose(tps[:], s2t_fm[:, sl], id16[:])
                nc.scalar.activation(s2t_lm[:, dsl], tps[:], AF.Copy)

            # ---------------- main loop over sequences ----------------
            for n in range(n_seq):
                # per-seq fp16 staging
                qt_st = sq_.tile([D, l_eff], F16, tag="qt_st")
                qtc_st = sq_.tile([D, l_eff], F16, tag="qtc_st")
                qts_st = sq_.tile([D, l_eff], F16, tag="qts_st")
                kf_st = sq_.tile([D, l_eff], F16, tag="kf_st")
                kcf_st = sq_.tile([D, l_eff], F16, tag="kcf_st")
                ksf_st = sq_.tile([D, l_eff], F16, tag="ksf_st")
                klm_st = sq_.tile([C, nch * D], F16, tag="klm_st")
                kcl_st = sq_.tile([C, nch * D], F16, tag="kcl_st")
                ksl_st = sq_.tile([C, nch * D], F16, tag="ksl_st")

                vw4_by_w = {}
                pend = []  # stashed (w, nfq) sin tails, flushed once per sequence
                last_exp = [None]

                def flush_sins():
                    from concourse.tile import add_dep_helper
                    for (pw, pnfq) in pend:
                        pwsl = bass.ds(pw * W, W)
                        sqw = win.tile([D, W], F16, tag="sqw")
                        nc.scalar.activation(sqw[:], pnfq[:], AF.Sin, scale=-TWO_PI)
                        sq2 = win.tile([D, W], F16, tag="sq2")
                        nc.vector.scalar_tensor_tensor(sq2[:], sqw[:], 0.5, sqw[:], OP.mult, OP.mult)
                        nc.vector.tensor_tensor(qt_st[:, pwsl], sq2[:], qel_st[:, pwsl], OP.mult)
                        nc.vector.scalar_tensor_tensor(qtc_st[:, pwsl], qt_st[:, pwsl], -1.0, c2t_fm[:, pwsl], OP.mult, OP.mult)
                        nc.vector.scalar_tensor_tensor(qts_st[:, pwsl], qt_st[:, pwsl], -1.0, s2t_fm[:, pwsl], OP.mult, OP.mult)
                    pend.clear()

                qel_st = sq_.tile([D, l_eff], F16, tag="qel_st")
                for w in range(nwin):
                    wsl = bass.ds(w * W, W)
                    wdl = bass.ds(w * cpw * D, cpw * D)  # lm-staging cols
                    # window loads: [128, cpw*D]
                    qw4 = win.tile([C, cpw * D], F32, tag="qw4")
                    nc.sync.dma_start(out=qw4[:], in_=q_ext[n, w * cpw : (w + 1) * cpw, :, :].rearrange("c p d -> p c d"))
                    kw4 = win.tile([C, cpw * D], F32, tag="kw4")
                    nc.sync.dma_start(out=kw4[:], in_=k_ext[n, w * cpw : (w + 1) * cpw, :, :].rearrange("c p d -> p c d"))
                    q2w4 = win.tile([C, cpw * D], F32, tag="q2w4")
                    nc.sync.dma_start(out=q2w4[:], in_=q2_ext[n, w * cpw : (w + 1) * cpw, :, :].rearrange("c p d -> p c d"))
                    vw4 = vwp.tile([C, cpw * D], F32, tag="vw4")
                    vw4_by_w[w] = vw4
                    nc.sync.dma_start(out=vw4[:], in_=v_ext[n, w * cpw : (w + 1) * cpw, :, :].rearrange("c p d -> p c d"))

                    # --- K path fully grouped in LM ---
                    rk = win.tile([C, cpw * D], F16, tag="rk")
                    nc.vector.tensor_scalar(rk[:], kw4[:], 0.0, None, OP.max)
                    mk = win.tile([C, cpw * D], F16, tag="mk")
                    nc.vector.tensor_scalar(mk[:], kw4[:], 0.0, None, OP.min)
                    ek = win.tile([C, cpw * D], F16, tag="ek")
                    nc.scalar.activation(ek[:], mk[:], AF.Exp)
                    nc.vector.tensor_tensor(klm_st[:, wdl], ek[:], rk[:], OP.add)
                    nc.vector.tensor_tensor(kcl_st[:, wdl], klm_st[:, wdl], c2t_lm[:, wdl], OP.mult)
                    nc.gpsimd.tensor_tensor(ksl_st[:, wdl], klm_st[:, wdl], s2t_lm[:, wdl], OP.mult)

                    kscr = drs.tile([W, D], F16, tag="kscr")
                    nc.sync.dma_start(out=kscr[:].rearrange("(c p) d -> p c d", c=cpw), in_=klm_st[:, wdl].rearrange("p (c d) -> p c d", c=cpw))
                    nc.sync.dma_start(out=kf_st[:, wsl], in_=kscr[:], transpose=True)

                    # --- Q path: per-chunk transposes, grouped recombine in FM ---
                    rq_w = win.tile([D, W], F16, tag="rq_w")
                    mq_w = win.tile([D, W], F16, tag="mq_w")
                    q2f = win.tile([D, W], F32, tag="q2f")
                    for cc in range(cpw):
                        gc = w * cpw + cc
                        lsl = bass.ds(cc * C, C)
                        tq = ptr.tile([D, C], F32, tag="tr")
                        nc.tensor.transpose(tq[:], qw4[:, bass.ds(cc * D, D)], id32[:])
                        nc.vector.tensor_scalar(rq_w[:, lsl], tq[:], 0.0, None, OP.max)
                        nc.vector.tensor_scalar(mq_w[:, lsl], tq[:], 0.0, None, OP.min)
                        tq2 = ptr.tile([D, C], F32, tag="tr")
                        nc.tensor.transpose(tq2[:], q2w4[:, bass.ds(cc * D, D)], id32[:])
                        nc.scalar.activation(q2f[:, lsl], tq2[:], AF.Copy)

                    eq = win.tile([D, W], F16, tag="eq")
                    eq_i = nc.scalar.activation(eq[:], mq_w[:], AF.Exp)
                    last_exp[0] = eq_i.ins
                    nc.vector.tensor_tensor(qel_st[:, wsl], eq[:], rq_w[:], OP.add)

                    # q2 projection; sin tail stashed, flushed per window pair
                    yp = pq2.tile([D, W], F32, tag="q2p")
                    nc.tensor.matmul(yp[:], omega_s[:], q2f[:], start=True, stop=True)
                    kq = win.tile([D, W], F32, tag="kq")
                    nc.scalar.activation(kq[:], yp[:], AF.Identity, bias=magic_col[:, 0:1])
                    nfq = nfp.tile([D, W], F32, tag="nfq")
                    nc.vector.scalar_tensor_tensor(nfq[:], kq[:], MAGIC, yp[:], OP.subtract, OP.subtract)
                    pend.append((w, nfq))
                    if len(pend) == 2 or w == nwin - 1:
                        flush_sins()

                    nc.vector.tensor_tensor(kcf_st[:, wsl], kf_st[:, wsl], c2t_fm[:, wsl], OP.mult)
                    nc.gpsimd.tensor_tensor(ksf_st[:, wsl], kf_st[:, wsl], s2t_fm[:, wsl], OP.mult)

                # ---- SCAN phase ----
                st1 = pS.tile([D, DV1], F32, tag="st1")
                stc = pS.tile([D, DV1], F32, tag="stc")
                sts = pS.tile([D, DV1], F32, tag="sts")
                ob4 = None
                for c in range(nch):
                    sl = bass.ts(c, C)
                    dsl = bass.ts(c, D)
                    first, last = c == 0, c == nch - 1
                    cc = c % cpw

                    vp = io.tile([C, DV1], F16, tag="vp")
                    nc.vector.tensor_copy(vp[:, 0:D], vw4_by_w[c // cpw][:, bass.ds((c % cpw) * D, D)])
                    nc.gpsimd.memset(vp[:, D:DV1], 1.0)

                    p_ps = pP.tile([C, C], F32, tag="P")
                    if first:
                        qa = wk.tile([D, C], F32, tag="qa")
                        nc.vector.scalar_tensor_tensor(qa[:], qt_st[:, 0:C], 0.5, s2_0[:], OP.mult, OP.mult)
                        qb = wk.tile([D, C], F32, tag="qb")
                        nc.vector.scalar_tensor_tensor(qb[:], qt_st[:, 0:C], 0.5, c2_0[:], OP.mult, OP.mult)
                        qc = wk.tile([D, C], F32, tag="qc")
                        nc.vector.scalar_tensor_tensor(qc[:], qt_st[:, 0:C], -1.0, sc_0[:], OP.mult, OP.mult)
                        ka = wk.tile([D, C], F32, tag="ka")
                        nc.vector.tensor_tensor(ka[:], kf_st[:, 0:C], c2_0[:], OP.mult)
                        kb = wk.tile([D, C], F32, tag="kb")
                        nc.vector.tensor_tensor(kb[:], kf_st[:, 0:C], s2_0[:], OP.mult)
                        kc = wk.tile([D, C], F32, tag="kc")
                        nc.vector.tensor_tensor(kc[:], kf_st[:, 0:C], sc_0[:], OP.mult)
                        nc.tensor.matmul(p_ps[:], ka[:], qa[:], start=True, stop=False)
                        nc.tensor.matmul(p_ps[:], kb[:], qb[:], start=False, stop=False)
                        nc.tensor.matmul(p_ps[:], kc[:], qc[:], start=False, stop=True)
                    else:
                        nc.tensor.matmul(p_ps[:], kf_st[:, sl], qt_st[:, sl], start=True, stop=False)
                        nc.tensor.matmul(p_ps[:], kcf_st[:, sl], qtc_st[:, sl], start=False, stop=False)
                        nc.tensor.matmul(p_ps[:], ksf_st[:, sl], qts_st[:, sl], start=False, stop=True)

                    p_sb = wk.tile([C, C], F16, tag="p_sb")
                    nc.vector.tensor_tensor(p_sb[:], p_ps[:], mask_sb[:], OP.mult)

                    o_ps = pO.tile([C, DV1], F32, tag="O")
                    nc.tensor.matmul(o_ps[:], p_sb[:], vp[:], start=True, stop=first)
                    if not first:
                        s1_sb = wk.tile([D, DV1], F16, tag="s1_sb")
                        nc.scalar.activation(s1_sb[:], st1[:], AF.Copy)
                        sc_sb = wk.tile([D, DV1], F16, tag="sc_sb")
                        nc.scalar.activation(sc_sb[:], stc[:], AF.Copy)
                        ss_sb = wk.tile([D, DV1], F16, tag="ss_sb")
                        nc.vector.tensor_copy(ss_sb[:], sts[:])
                        nc.tensor.matmul(o_ps[:], qt_st[:, sl], s1_sb[:], start=False, stop=False)
                        nc.tensor.matmul(o_ps[:], qtc_st[:, sl], sc_sb[:], start=False, stop=False)
                        nc.tensor.matmul(o_ps[:], qts_st[:, sl], ss_sb[:], start=False, stop=True)

                    if not last:
                        nc.tensor.matmul(st1[:], klm_st[:, dsl], vp[:], start=first, stop=True, skip_group_check=not first)
                        nc.tensor.matmul(stc[:], kcl_st[:, dsl], vp[:], start=first, stop=True, skip_group_check=not first)
                        nc.tensor.matmul(sts[:], ksl_st[:, dsl], vp[:], start=first, stop=True, skip_group_check=not first)

                    zc = op_.tile([C, 1], F32, tag="zc")
                    nc.vector.tensor_scalar(zc[:], o_ps[:, D:DV1], EPS, None, OP.add)
                    rz = op_.tile([C, 1], F32, tag="rz")
                    nc.vector.reciprocal(rz[:], zc[:])
                    if cc == 0:
                        ob4 = op_.tile([C, cpw * D], F32, tag="ob4")
                    nc.scalar.activation(ob4[:, bass.ds(cc * D, D)], o_ps[:, 0:D], AF.Copy, scale=rz[:, 0:1])
                    if cc == cpw - 1:
                        w0 = c // cpw
                        nc.sync.dma_start(
                            out=out_ext[n, w0 * cpw : (w0 + 1) * cpw, :, :].rearrange("c p d -> p c d"),
                            in_=ob4[:],
                        )

    nc.finalize()
    return nc


def _host_inputs(inputs, n_seq=N, nch=NCH):
    l_eff = nch * C
    q = np.ascontiguousarray(inputs["queries"], dtype=np.float32)
    q2 = np.ascontiguousarray(inputs["q2"], dtype=np.float32)
    k = np.ascontiguousarray(inputs["keys"], dtype=np.float32)
    v = np.ascontiguousarray(inputs["values"], dtype=np.float32)
    om = np.ascontiguousarray(inputs["omega"], dtype=np.float32)

    mask = np.triu(np.ones((C, C), dtype=np.float16))
    pos2pi = np.broadcast_to(
        (np.arange(l_eff, dtype=np.float64) / L / (2.0 * np.pi)).astype(np.float32)[None, :],
        (D, l_eff),
    ).copy()

    def shp(x, h):
        return np.ascontiguousarray(x[:n_seq, :l_eff, h, :]).reshape(n_seq, nch, C, D)

    in_maps = []
    for h in range(om.shape[0] if om.ndim == 3 else H):
        in_maps.append(
            {
                "queries": shp(q, h),
                "q2": shp(q2, h),
                "keys": shp(k, h),
                "values": shp(v, h),
                "omega": np.ascontiguousarray(om[h]),
                "mask": mask,
                "pos2pi": pos2pi,
            }
        )
    return in_maps


def _run(inputs, trace=False):
    if "nc" not in _CACHE:
        _CACHE["nc"] = build_nc()
    nc = _CACHE["nc"]
    in_maps = _host_inputs(inputs)
    res = run_bass_kernel_spmd(nc, in_maps, core_ids=list(range(H)), trace=trace)
    outs = [res.results[hh]["out"].reshape(N, L, D) for hh in range(H)]
    full = np.stack(outs, axis=2)
    return full.astype(np.float32), res


def kernel(**inputs):
    out, _ = _run(inputs, trace=False)
    return out

